# revision 9
# baseline (speedup 1.0000x reference)
"""Trainium2 Bass kernel for the LogicLayer problem, v4 (PE-scatter).

Like v3 (trail chaining: chunk t's A-operand reuses chunk t-1's B-gather),
but trail-piece starts ("breaks") may sit at ARBITRARY partitions: the fix
rows are gathered densely into prefix partitions of the fix slot and routed
to their target partitions by the otherwise-idle TensorE:

    A_psum = Diag(continue_mask) @ B_prev  +  Onehot @ Fix      (PSUM, f32)

The fused DVE ops read A straight from PSUM:

    V   = (k3*A + k2) * B        (affine_mul_reduce)
    OUT = (k1*A + k0) + V        (affine_then_add)

This removes the nested-break-chain constraint of v3 entirely, so packing is
a trivial exact bin-fill: pieces/core ~= #trails/core (~620) instead of
~1600, i.e. ~22 MiB of gather reads per core instead of 29-32 MiB.

The break schedule capacity M_t is shared across cores (SPMD); cores with
fewer breaks pad the gather with junk index 0 whose onehot column is zero.
"""

import numpy as np

_GATE_COEFFS = np.array(
    [
        [0.0, 0.0, 0.0, 0.0],
        [0.0, 0.0, 0.0, 1.0],
        [0.0, 1.0, 0.0, -1.0],
        [0.0, 1.0, 0.0, 0.0],
        [0.0, 0.0, 1.0, -1.0],
        [0.0, 0.0, 1.0, 0.0],
        [0.0, 1.0, 1.0, -2.0],
        [0.0, 1.0, 1.0, -1.0],
        [1.0, -1.0, -1.0, 1.0],
        [1.0, -1.0, -1.0, 2.0],
        [1.0, 0.0, -1.0, 0.0],
        [1.0, 0.0, -1.0, 1.0],
        [1.0, -1.0, 0.0, 0.0],
        [1.0, -1.0, 0.0, 1.0],
        [1.0, 0.0, 0.0, -1.0],
        [1.0, 0.0, 0.0, 0.0],
    ],
    dtype=np.float32,
)
_SIGMA = np.array([0, 1, 4, 5, 2, 3, 6, 7, 8, 9, 12, 13, 10, 11, 14, 15])

BATCH, IN_DIM, OUT_DIM = 4096, 16384, 16384
NCORES = 8
OC = OUT_DIM // NCORES
NCHUNK = OC // 128
HALF = BATCH // 2

_PROGRAM_CACHE: dict = {}
_PLAN_CACHE: dict = {}


def _decompose_trails(a_idx, b_idx):
    n_edges = len(a_idx)
    adj: dict = {}
    for i in range(n_edges):
        u = int(a_idx[i]); v = int(b_idx[i])
        adj.setdefault(u, []).append((i, v))
        adj.setdefault(v, []).append((i, u))
    used = np.zeros(n_edges, dtype=bool)
    ptr = {u: 0 for u in adj}
    deg = {u: len(lst) for u, lst in adj.items()}

    def walk(start):
        trail = []
        cur = start
        while True:
            lst = adj[cur]
            p = ptr[cur]
            while p < len(lst) and used[lst[p][0]]:
                p += 1
            ptr[cur] = p
            if p >= len(lst):
                break
            eid, nxt = lst[p]
            used[eid] = True
            trail.append((eid, cur, nxt))
            cur = nxt
        return trail

    trails = []
    order = [u for u in adj if deg[u] % 2 == 1] + [
        u for u in adj if deg[u] % 2 == 0
    ]
    for u in order:
        while True:
            t = walk(u)
            if not t:
                break
            trails.append(t)
    assert sum(len(t) for t in trails) == n_edges
    return trails


def _fill_rows(pieces):
    """Exact bin-fill of pieces (lists of edges, each len<=16) into 128 rows
    of total length 16.  Returns rows as lists of pieces."""
    buckets = [[] for _ in range(17)]
    for pc in pieces:
        buckets[len(pc)].append(pc)
    rows = []
    for _ in range(128):
        rem = 16
        row = []
        while rem > 0:
            # largest piece <= rem, else cut the largest piece
            l = 0
            for L in range(rem, 0, -1):
                if buckets[L]:
                    l = L
                    break
            if l == 0:
                Lmax = max(L for L in range(17) if buckets[L])
                pc = buckets[Lmax].pop()
                row.append(pc[:rem])
                buckets[Lmax - rem].append(pc[rem:])
                rem = 0
            else:
                row.append(buckets[l].pop())
                rem -= l
        rows.append(row)
    assert not any(buckets[L] for L in range(17))
    return rows


def _order_rows(rows_pieces, col_load):
    """Order pieces within each row to balance break counts per column.
    Greedy: at each position, pick the piece whose end lands on the
    least-loaded next column. Mutates col_load (16 counts, col 0 ignored)."""
    rows = []
    for pieces in rows_pieces:
        remaining = list(pieces)
        order = []
        pos = 0
        while remaining:
            best = None
            bestload = None
            for i, pc in enumerate(remaining):
                nxt = pos + len(pc)
                load = col_load[nxt] if nxt < 16 else -1  # prefer finishing
                if bestload is None or load < bestload:
                    bestload = load
                    best = i
            pc = remaining.pop(best)
            order.append(pc)
            pos += len(pc)
            if pos < 16:
                col_load[pos] += 1
        rows.append(order)
    return rows


def _build_plan(a_idx, b_idx):
    a = np.asarray(a_idx).astype(np.int64)
    b = np.asarray(b_idx).astype(np.int64)
    trails = _decompose_trails(a, b)

    # cut to <=16 and split across cores with balanced piece counts:
    # round-robin assignment of pieces (largest first) under the 2048-edge cap
    pieces_all = []
    for t in trails:
        for i in range(0, len(t), 16):
            pieces_all.append(t[i : i + 16])
    pieces_all.sort(key=len, reverse=True)
    core_fill = [0] * NCORES
    core_pieces = [[] for _ in range(NCORES)]
    for pc in pieces_all:
        # most-empty core that still fits; else cut to fit the most-empty
        order = sorted(range(NCORES), key=lambda c: core_fill[c])
        placed = False
        for c in order:
            if core_fill[c] + len(pc) <= OC:
                core_pieces[c].append(pc)
                core_fill[c] += len(pc)
                placed = True
                break
        if not placed:
            c = order[0]
            room = OC - core_fill[c]
            assert room > 0
            core_pieces[c].append(pc[:room])
            core_fill[c] += room
            pc2 = pc[room:]
            c2 = min(range(NCORES), key=lambda cc: core_fill[cc])
            assert core_fill[c2] + len(pc2) <= OC
            core_pieces[c2].append(pc2)
            core_fill[c2] += len(pc2)
    assert all(f == OC for f in core_fill)

    grids = []        # grids[c][p][t] = (y, frm, to)
    breaks = []       # breaks[c][p][t] = bool (piece starts at (p,t))
    m_cores = np.zeros((NCORES, NCHUNK), dtype=np.int64)
    for c in range(NCORES):
        rows_p = _fill_rows(core_pieces[c])
        col_load = [0] * NCHUNK
        rows_p = _order_rows(rows_p, col_load)
        grid = []
        brk = []
        for p in range(128):
            edges = []
            bk = [False] * NCHUNK
            for pc in rows_p[p]:
                bk[len(edges)] = True
                edges.extend(pc)
            assert len(edges) == NCHUNK
            grid.append(edges)
            brk.append(bk)
            for t in range(NCHUNK):
                if bk[t]:
                    m_cores[c, t] += 1
        grids.append(grid)
        breaks.append(brk)

    M = [int(m_cores[:, t].max()) for t in range(NCHUNK)]
    M[0] = 128
    reg = [128 + M[t] for t in range(NCHUNK)]
    ni = [-(-r // 16) * 16 for r in reg]
    cols = [n // 16 for n in ni]

    ia_cores = []
    oh_cores = []
    dg_cores = []
    y_of_row = np.empty((NCORES, OC), dtype=np.int64)
    swap = np.zeros((NCORES, 128, NCHUNK), dtype=bool)
    for c in range(NCORES):
        grid = grids[c]
        brk = breaks[c]
        seqs = []
        oh = np.zeros((128, NCHUNK, 128), dtype=np.float16)  # [q, t, p]
        dg = np.zeros((128, NCHUNK, 128), dtype=np.float16)
        for t in range(NCHUNK):
            seq = np.full(ni[t], -1, dtype=np.int16)
            # chunk 0: fixes ordered by partition (identity), read directly
            fixq = 0
            for p in range(128):
                y, frm, to = grid[p][t]
                assert (frm == a[y] and to == b[y]) or (
                    frm == b[y] and to == a[y]
                ), (c, p, t)
                seq[p] = to
                y_of_row[c, t * 128 + p] = y
                swap[c, p, t] = frm == b[y] and to == a[y] and a[y] != b[y]
                if brk[p][t]:
                    seq[128 + fixq] = frm
                    oh[fixq, t, p] = 1.0
                    fixq += 1
                else:
                    assert t >= 1 and frm == grid[p][t - 1][2], (c, p, t)
                    dg[p, t, p] = 1.0
            assert fixq == m_cores[c, t]
            # junk-pad the fix block up to the shared M_t with index 0
            seq[128 + fixq : 128 + M[t]] = 0
            seqs.append(seq)
        flatseq = np.concatenate(seqs)
        w = np.ascontiguousarray(flatseq.reshape(-1, 16).T)
        ia_cores.append(np.ascontiguousarray(np.tile(w, (8, 1))))
        oh_cores.append(np.ascontiguousarray(oh.reshape(128, NCHUNK * 128)))
        dg_cores.append(np.ascontiguousarray(dg.reshape(128, NCHUNK * 128)))
    return {
        "m": tuple(M),
        "m_cores": m_cores,
        "reg": reg,
        "ni": ni,
        "cols": cols,
        "ia_cores": ia_cores,
        "oh_cores": oh_cores,
        "dg_cores": dg_cores,
        "y_of_row": y_of_row,
        "swap": swap,
    }


def _get_plan(a_idx, b_idx):
    key = (np.asarray(a_idx).tobytes(), np.asarray(b_idx).tobytes())
    h = hash(key)
    if _PLAN_CACHE.get("key") != h:
        _PLAN_CACHE["key"] = h
        _PLAN_CACHE["plan"] = _build_plan(a_idx, b_idx)
    return _PLAN_CACHE["plan"]


def _build_program(m, ni, reg, cols):
    import concourse.bass as bass  # noqa: F401
    import concourse.tile as tile
    from concourse import bacc, mybir

    f32 = mybir.dt.float32
    f16 = mybir.dt.float16
    i16 = mybir.dt.int16

    total_cols = sum(cols)
    nc = bacc.Bacc("TRN2", target_bir_lowering=False, debug=False)
    xT_h = nc.dram_tensor("xT", [IN_DIM, BATCH], f16, kind="ExternalInput")
    ia_h = nc.dram_tensor("ia", [128, total_cols], i16, kind="ExternalInput")
    kg_h = nc.dram_tensor("kg", [128, 4 * NCHUNK], f32, kind="ExternalInput")
    oh_h = nc.dram_tensor("oh", [128, NCHUNK * 128], f16, kind="ExternalInput")
    dg_h = nc.dram_tensor("dg", [128, NCHUNK * 128], f16, kind="ExternalInput")
    out_h = nc.dram_tensor("outT", [OC, BATCH], f16, kind="ExternalOutput")

    with tile.TileContext(nc) as tc:
        from contextlib import ExitStack

        with ExitStack() as stack:
            cp = stack.enter_context(tc.tile_pool(name="const", bufs=1))

            ia_sb = cp.tile([128, total_cols], i16)
            nc.sync.dma_start(ia_sb[:], ia_h.ap()[:, :])
            kg_sb = cp.tile([128, 4 * NCHUNK], f32, tag="kg")
            nc.sync.dma_start(kg_sb[:], kg_h.ap()[:, :])
            oh_sb = cp.tile([128, NCHUNK, 128], f16, tag="oh")
            nc.sync.dma_start(
                oh_sb[:], oh_h.ap().rearrange("p (t q) -> p t q", t=NCHUNK)
            )
            dg_sb = cp.tile([128, NCHUNK, 128], f16, tag="dg")
            nc.sync.dma_start(
                dg_sb[:], dg_h.ap().rearrange("p (t q) -> p t q", t=NCHUNK)
            )
            zi = cp.tile([128, 1], i16, tag="zi")
            nc.gpsimd.memset(zi[:], 0)
            warm = cp.tile([128, 1, BATCH], f16, tag="warm")

            outT_ap = out_h.ap().rearrange("(c p) n -> p c n", p=128)
            with (
                tc.tile_pool(name="pg", bufs=4) as pg,
                tc.tile_pool(name="po", bufs=3) as po,
                tc.tile_pool(name="pv", bufs=3) as pv,
                tc.tile_pool(name="pa", bufs=3) as pa,
                tc.tile_pool(name="ps", bufs=2, space="PSUM") as ps,
            ):
                nc.gpsimd.dma_gather(
                    out_ap=warm[:, 0:1, :],
                    in_ap=xT_h.ap()[:, :],
                    idxs_ap=zi[:],
                    num_idxs=16,
                    num_idxs_reg=16,
                    elem_size=BATCH,
                    single_packet=False,
                )
                G_prev = None
                c0 = 0
                for t in range(NCHUNK):
                    G = pg.tile([128, 2, BATCH], f16, tag="G")
                    gout = G[:, 0:1, :] if ni[t] <= 128 else G[:]
                    nc.gpsimd.dma_gather(
                        out_ap=gout,
                        in_ap=xT_h.ap()[:, :],
                        idxs_ap=ia_sb[:, c0 : c0 + cols[t]],
                        num_idxs=ni[t],
                        num_idxs_reg=reg[t],
                        elem_size=BATCH,
                        single_packet=False,
                    )
                    c0 += cols[t]

                    O = po.tile([128, 1, BATCH], f16, tag="O")
                    V = pv.tile([128, BATCH], f16, tag="V")
                    acc = pa.tile([128, 1], f32, tag="acc")
                    P01 = [None, None]
                    if t >= 1 and m[t] > 0:
                        P01 = [
                            ps.tile([128, HALF], f32, tag="P", name=f"P{t}h{h}")
                            for h in range(2)
                        ]
                        # dg pass over all 8 banks (one lhsT load), then oh
                        # pass per half so half 0 is consumable early
                        for h in range(2):
                            for q in range(4):
                                cq = slice(h * HALF + q * 512, h * HALF + (q + 1) * 512)
                                nc.tensor.matmul(
                                    P01[h][:, q * 512 : (q + 1) * 512],
                                    dg_sb[:, t, :],
                                    G_prev[:, 0, cq],
                                    start=True,
                                    stop=False,
                                )
                        for h in range(2):
                            for q in range(4):
                                cq = slice(h * HALF + q * 512, h * HALF + (q + 1) * 512)
                                # contract only over the M_t written fix rows;
                                # partitions above them are uninitialized SBUF
                                nc.tensor.matmul(
                                    P01[h][:, q * 512 : (q + 1) * 512],
                                    oh_sb[0 : m[t], t, :],
                                    G[0 : m[t], 1, cq],
                                    start=False,
                                    stop=True,
                                )
                    for h in range(2):
                        cs = slice(h * HALF, (h + 1) * HALF)
                        if t == 0:
                            A = G[:, 1, cs]     # identity-ordered fixes
                        elif m[t] == 0:
                            A = G_prev[:, 0, cs]
                        else:
                            A = P01[h][:]
                        nc.vector.affine_mul_reduce(
                            out=V[:, cs],
                            accum_out=acc[:],
                            in0=A,
                            in1=G[:, 0, cs],
                            scale=kg_sb[:, 3 * NCHUNK + t : 3 * NCHUNK + t + 1],
                            bias=kg_sb[:, 2 * NCHUNK + t : 2 * NCHUNK + t + 1],
                        )
                        nc.vector.affine_then_add(
                            out=O[:, 0, cs],
                            in0=A,
                            in1=V[:, cs],
                            scale=kg_sb[:, 1 * NCHUNK + t : 1 * NCHUNK + t + 1],
                            bias=kg_sb[:, 0 * NCHUNK + t : 0 * NCHUNK + t + 1],
                        )
                    nc.sync.dma_start(outT_ap[:, t : t + 1, :], O[:, :, :])
                    G_prev = G

    nc.compile()
    return nc


def _host_inputs(x, weights, a_idx, b_idx):
    plan = _get_plan(a_idx, b_idx)
    weights = np.asarray(weights, dtype=np.float32)
    xT16 = np.ascontiguousarray(
        np.asarray(x, dtype=np.float32).T.astype(np.float16)
    )
    w = weights - weights.max(axis=1, keepdims=True)
    e = np.exp(w)
    P = e / e.sum(axis=1, keepdims=True)
    K0 = P @ _GATE_COEFFS
    K1 = P @ _GATE_COEFFS[_SIGMA]
    y_of_row = plan["y_of_row"]
    swap = plan["swap"]
    in_maps = []
    for c in range(NCORES):
        yr = y_of_row[c].reshape(NCHUNK, 128)
        kc = np.where(
            swap[c].transpose(1, 0)[:, :, None], K1[yr], K0[yr]
        )
        kg = np.ascontiguousarray(
            kc.transpose(1, 2, 0).reshape(128, 4 * NCHUNK)
        ).astype(np.float32)
        in_maps.append(
            {
                "xT": xT16,
                "ia": plan["ia_cores"][c],
                "kg": kg,
                "oh": plan["oh_cores"][c],
                "dg": plan["dg_cores"][c],
            }
        )
    return in_maps


def kernel(x, weights, a_idx, b_idx):
    from concourse.bass_utils import run_bass_kernel_spmd

    plan = _get_plan(a_idx, b_idx)
    pkey = plan["m"]
    if _PROGRAM_CACHE.get("mkey") != pkey:
        _PROGRAM_CACHE["mkey"] = pkey
        _PROGRAM_CACHE["nc"] = _build_program(
            plan["m"], plan["ni"], plan["reg"], plan["cols"]
        )
    nc = _PROGRAM_CACHE["nc"]

    in_maps = _host_inputs(x, weights, a_idx, b_idx)
    res = run_bass_kernel_spmd(nc, in_maps, list(range(NCORES)))
    outT = np.concatenate(
        [res.results[c]["outT"] for c in range(NCORES)], axis=0
    )
    y_all = plan["y_of_row"].reshape(-1)
    full = np.empty_like(outT)
    full[y_all] = outT
    return np.ascontiguousarray(full.T).astype(np.float32)
